# revision 5
# baseline (speedup 1.0000x reference)
"""JointAngleLoss Trainium2 kernel v7 (8-core data-parallel).

Engine schedule per group (K=128, G=4):
  SP  : 3 DMA chunks of x [P, 63K] fp32 (un-duplicated layout [c][j][k])
  ACT : dedup cast fp32->fp16 into 75-slot layout [c][jj][f][k]
        (+ optionally the bones slab-dup copy)
  DVE : bones (3 contiguous subs), [dup copy], m1/m2 (c-fused 3D or per-c),
        rot, pp (fused), vt, v, relu via dual-op tensor_scalar
  PE  : coplanarity trace matmuls (lhsT=b4 block, rhs=[palm;mid]) and
        optionally the relu^2 sum as mrelu x mrelu trace matmuls
Host sums PSUM diagonals + accum columns in float64.
"""
import sys

for _p in ("/opt/trn_rl_repo", "/root/.axon_site/_ro/trn_rl_repo"):
    if _p not in sys.path:
        sys.path.append(_p)

import numpy as np

import concourse.bacc as bacc
import concourse.mybir as mybir
from concourse import tile
from concourse.ap import AP
from concourse.alu_op_type import AluOpType
from concourse.bass_utils import run_bass_kernel_spmd
from contextlib import ExitStack

N_CORES = 8
P = 128
B_FULL = 524288

F16 = mybir.dt.float16
F32 = mybir.dt.float32
ACTF = mybir.ActivationFunctionType


def _ap(base, off, dims):
    return AP(base.tensor, base.offset + off, [list(base.ap[0])] + dims)


def build_v7(rows_per_core, K=128, reps=1, hw_loop=1, mode="full",
             cross="perc", copy34_eng="scalar", relsq="pe",
             dma_chunks=3, bones_bufs=2, rot_bufs=1, xh_bufs=1, bones_fused=False):
    assert rows_per_core % (P * K) == 0
    G = rows_per_core // (P * K)
    CJ = 21 * K
    FK = 63 * K
    SK = 25 * K
    S5 = 5 * K
    CB = 20 * K
    MR = 9 * S5
    PPN = 6 * S5
    NV = 2 * S5
    NB = S5 // 128
    NVB = NV // 128
    assert S5 % 128 == 0

    nc = bacc.Bacc("TRN2", target_bir_lowering=False, debug=False)

    x = nc.dram_tensor("x", [G, P, FK], F32, kind="ExternalInput")
    ncols = 384 if relsq == "pe" else 256
    cop_out = nc.dram_tensor("cop_out", [P, ncols], F32, kind="ExternalOutput")
    mask_out = nc.dram_tensor("mask_out", [P, G * reps], F32, kind="ExternalOutput")

    with tile.TileContext(nc) as tc, ExitStack() as ctx:
        xpool = ctx.enter_context(tc.tile_pool(name="xpool", bufs=2))
        hpool = ctx.enter_context(tc.tile_pool(name="hpool", bufs=xh_bufs))
        bpool = ctx.enter_context(tc.tile_pool(name="bpool", bufs=bones_bufs))
        mpool = ctx.enter_context(tc.tile_pool(name="mpool", bufs=1))
        rpool = ctx.enter_context(tc.tile_pool(name="rpool", bufs=rot_bufs))
        vpool = ctx.enter_context(tc.tile_pool(name="vpool", bufs=1))
        spool = ctx.enter_context(tc.tile_pool(name="spool", bufs=1))
        psum = ctx.enter_context(tc.tile_pool(name="psum", bufs=1, space="PSUM"))

        acc = spool.tile([P, G * reps], F32)
        psum_M = psum.tile([P, 256], F32)
        psum_M2 = psum.tile([P, 128], F32, name="psum_M2") if relsq == "pe" else None
        if relsq != "act":
            nc.gpsimd.memset(acc[:], 0.0)

        xt_static = None
        if mode == "nodma":
            xt_static = spool.tile([P, FK], F32)
            nc.gpsimd.memset(xt_static[:], 0.5)

        loop_cm = tc.For_i(0, hw_loop, 1) if hw_loop > 1 else None
        if loop_cm is not None:
            loop_cm.__enter__()

        for rep in range(reps):
            for g in range(G):
                first = rep == 0 and g == 0
                last = rep == reps - 1 and g == G - 1

                xt = xt_static if mode == "nodma" else xpool.tile([P, FK], F32)
                if mode != "nodma":
                    for ch in range(dma_chunks):
                        lo = FK * ch // dma_chunks
                        hi = FK * (ch + 1) // dma_chunks
                        nc.sync.dma_start(xt[:, lo:hi], x.ap()[g][:, lo:hi])
                if mode == "dma":
                    sink = vpool.tile([P, 2], F32, tag="sink")
                    nc.scalar.activation(sink[:], xt[:, 0:2], ACTF.Copy)
                    continue

                xh = hpool.tile([P, 3 * SK], F16)
                xtb = xt[:]
                for c in range(3):
                    src = _ap(xtb, c * CJ, [[K, 5], [4 * K, 5], [1, K]])
                    dst = xh[:, c * SK:(c + 1) * SK].rearrange(
                        "p (jj f k) -> p jj f k", jj=5, f=5, k=K)
                    nc.scalar.activation(dst, src, ACTF.Copy)

                nslab = 5 if cross == "fused" else 3
                bones = bpool.tile([P, nslab * CB], F16, tag="bones")
                if bones_fused:
                    sd = [[SK, 3], [1, CB]]
                    nc.vector.tensor_sub(
                        bones[:, 0:3 * CB].rearrange("p (c e) -> p c e", c=3, e=CB),
                        _ap(xh[:], S5, sd), _ap(xh[:], 0, sd))
                else:
                    for s in range(3):
                        nc.vector.tensor_sub(
                            bones[:, s * CB:(s + 1) * CB],
                            xh[:, s * SK + S5: s * SK + SK],
                            xh[:, s * SK: s * SK + CB])

                bb = bones[:]
                m1 = mpool.tile([P, MR], F16, tag="m1")
                m2 = mpool.tile([P, MR], F16, tag="m2")
                rot = rpool.tile([P, MR], F16, tag="rot")
                if cross == "fused":
                    if copy34_eng == "scalar":
                        nc.scalar.copy(bones[:, 3 * CB:5 * CB], bones[:, 0:2 * CB])
                    else:
                        nc.vector.tensor_copy(bones[:, 3 * CB:5 * CB],
                                              bones[:, 0:2 * CB])
                    cdims = [[CB, 3], [S5, 3], [1, S5]]
                    nc.vector.tensor_mul(
                        m1[:].rearrange("p (c q e) -> p c q e", c=3, q=3, e=S5),
                        _ap(bb, CB + S5, cdims), _ap(bb, 2 * CB, cdims))
                    nc.vector.tensor_mul(
                        m2[:].rearrange("p (c q e) -> p c q e", c=3, q=3, e=S5),
                        _ap(bb, 2 * CB + S5, cdims), _ap(bb, CB, cdims))
                else:
                    qd = [[S5, 3], [1, S5]]
                    for c in range(3):
                        c1, c2 = (c + 1) % 3, (c + 2) % 3
                        nc.vector.tensor_mul(
                            m1[:, c * 3 * S5:(c + 1) * 3 * S5].rearrange(
                                "p (q e) -> p q e", q=3, e=S5),
                            _ap(bb, c1 * CB + S5, qd), _ap(bb, c2 * CB, qd))
                    for c in range(3):
                        c1, c2 = (c + 1) % 3, (c + 2) % 3
                        nc.vector.tensor_mul(
                            m2[:, c * 3 * S5:(c + 1) * 3 * S5].rearrange(
                                "p (q e) -> p q e", q=3, e=S5),
                            _ap(bb, c2 * CB + S5, qd), _ap(bb, c1 * CB, qd))
                nc.vector.tensor_sub(rot[:], m1[:], m2[:])

                rb = rot[:]
                for c in range(3):
                    for jb in range(NB):
                        lhsT = bones[:, c * CB + 3 * S5 + jb * 128:
                                     c * CB + 3 * S5 + (jb + 1) * 128]
                        rhs = _ap(rb, c * 3 * S5 + jb * 128, [[S5, 2], [1, 128]])
                        nc.tensor.matmul(
                            psum_M[:], lhsT, rhs,
                            start=(first and c == 0 and jb == 0),
                            stop=(last and c == 2 and jb == NB - 1))

                pp = vpool.tile([P, PPN], F16, tag="pp", bufs=2)
                nc.vector.tensor_mul(
                    pp[:].rearrange("p (h c e) -> p h c e", h=2, c=3, e=S5),
                    _ap(rb, 2 * S5, [[-2 * S5, 2], [3 * S5, 3], [1, S5]]),
                    _ap(rb, S5, [[0, 2], [3 * S5, 3], [1, S5]]))

                pb = pp[:]
                vt = vpool.tile([P, NV], F16, tag="vt")
                v = vpool.tile([P, NV], F16, tag="v", bufs=2)
                hdims = [[3 * S5, 2], [1, S5]]
                v2d = lambda t: t[:].rearrange("p (h e) -> p h e", h=2, e=S5)
                nc.vector.tensor_add(v2d(vt), _ap(pb, 0, hdims), _ap(pb, S5, hdims))
                nc.vector.tensor_add(v2d(v), v2d(vt), _ap(pb, 2 * S5, hdims))

                col = rep * G + g
                if relsq == "act":
                    mrelu = vpool.tile([P, NV], F16, tag="mrelu")
                    sqj = vpool.tile([P, NV], F16, tag="sqj")
                    nc.scalar.activation(mrelu[:], v[:], ACTF.Relu, scale=-1.0)
                    nc.scalar.activation(sqj[:], mrelu[:], ACTF.Square,
                                         accum_out=acc[:, col:col + 1])
                elif relsq == "dve":
                    mrelu = vpool.tile([P, NV], F16, tag="mrelu")
                    sqj = vpool.tile([P, NV], F16, tag="sqj")
                    nc.vector.tensor_scalar(mrelu[:], v[:], -1.0, 0.0,
                                            AluOpType.mult, AluOpType.max)
                    nc.vector.tensor_tensor_reduce(
                        sqj[:], mrelu[:], mrelu[:], 1.0, 0.0,
                        AluOpType.mult, AluOpType.add,
                        accum_out=acc[:, col:col + 1])
                else:  # pe
                    mrelu = vpool.tile([P, NV], F16, tag="mrelu", bufs=2)
                    nc.vector.tensor_scalar(mrelu[:], v[:], -1.0, 0.0,
                                            AluOpType.mult, AluOpType.max)
                    for vb in range(NVB):
                        blk = mrelu[:, vb * 128:(vb + 1) * 128]
                        nc.tensor.matmul(
                            psum_M2[:], blk, blk,
                            start=(first and vb == 0),
                            stop=(last and vb == NVB - 1))

        if loop_cm is not None:
            loop_cm.__exit__(None, None, None)

        cop_sb = spool.tile([P, ncols], F32)
        if mode != "dma":
            nc.scalar.copy(cop_sb[:, 0:256], psum_M[:])
            if psum_M2 is not None:
                nc.scalar.copy(cop_sb[:, 256:384], psum_M2[:])
        else:
            nc.gpsimd.memset(cop_sb[:], 0.0)
        nc.sync.dma_start(cop_out.ap(), cop_sb[:])
        nc.sync.dma_start(mask_out.ap(), acc[:])

    nc.compile()
    return nc, G


def host_planarize63(x: np.ndarray, n_cores: int, K: int) -> np.ndarray:
    B = x.shape[0]
    R = B // n_cores
    G = R // (P * K)
    xr = x.reshape(n_cores, G, P, K, 21, 3)
    xp = xr.transpose(0, 1, 2, 5, 4, 3)
    return np.ascontiguousarray(xp).reshape(n_cores, G, P, 63 * K)


_CACHE = {}


def _get_nc(rows_per_core: int, K: int):
    key = (rows_per_core, K)
    if key not in _CACHE:
        _CACHE[key] = build_v7(rows_per_core, K)
    return _CACHE[key]


def kernel(pose23d_pred: np.ndarray) -> np.ndarray:
    x = np.asarray(pose23d_pred, dtype=np.float32)
    assert x.shape == (B_FULL, 21, 3), x.shape
    K = 128
    R = B_FULL // N_CORES
    nc, G = _get_nc(R, K)
    xp = host_planarize63(x, N_CORES, K)
    in_maps = [{"x": xp[i]} for i in range(N_CORES)]
    res = run_bass_kernel_spmd(nc, in_maps, list(range(N_CORES)))
    total = 0.0
    for r in res.results:
        M = r["cop_out"].astype(np.float64)
        total += np.trace(M[:, 0:128]) + np.trace(M[:, 128:256])
        if M.shape[1] >= 384:
            total += np.trace(M[:, 256:384])
        total += r["mask_out"].astype(np.float64).sum()
    return np.float32(total)


# revision 6
# speedup vs baseline: 1.0654x; 1.0654x over previous
"""JointAngleLoss Trainium2 kernel v7 (8-core data-parallel).

Engine schedule per group (K=128, G=4):
  SP  : 3 DMA chunks of x [P, 63K] fp32 (un-duplicated layout [c][j][k])
  ACT : dedup cast fp32->fp16 into 75-slot layout [c][jj][f][k]
        (+ optionally the bones slab-dup copy)
  DVE : bones (3 contiguous subs), [dup copy], m1/m2 (c-fused 3D or per-c),
        rot, pp (fused), vt, v, relu via dual-op tensor_scalar
  PE  : coplanarity trace matmuls (lhsT=b4 block, rhs=[palm;mid]) and
        optionally the relu^2 sum as mrelu x mrelu trace matmuls
Host sums PSUM diagonals + accum columns in float64.
"""
import sys

for _p in ("/opt/trn_rl_repo", "/root/.axon_site/_ro/trn_rl_repo"):
    if _p not in sys.path:
        sys.path.append(_p)

import numpy as np

import concourse.bacc as bacc
import concourse.mybir as mybir
from concourse import tile
from concourse.ap import AP
from concourse.alu_op_type import AluOpType
from concourse.bass_utils import run_bass_kernel_spmd
from contextlib import ExitStack

N_CORES = 8
P = 128
B_FULL = 524288

F16 = mybir.dt.float16
F32 = mybir.dt.float32
ACTF = mybir.ActivationFunctionType


def _ap(base, off, dims):
    return AP(base.tensor, base.offset + off, [list(base.ap[0])] + dims)


def build_v7(rows_per_core, K=128, reps=1, hw_loop=1, mode="full",
             cross="perc", copy34_eng="scalar", relsq="pe",
             dma_chunks=3, bones_bufs=2, rot_bufs=1, xh_bufs=1, bones_fused=False):
    assert rows_per_core % (P * K) == 0
    G = rows_per_core // (P * K)
    CJ = 21 * K
    FK = 63 * K
    SK = 25 * K
    S5 = 5 * K
    CB = 20 * K
    MR = 9 * S5
    PPN = 6 * S5
    NV = 2 * S5
    NB = S5 // 128
    NVB = NV // 128
    assert S5 % 128 == 0

    nc = bacc.Bacc("TRN2", target_bir_lowering=False, debug=False)

    x = nc.dram_tensor("x", [G, P, FK], F32, kind="ExternalInput")
    ncols = 384 if relsq == "pe" else 256
    cop_out = nc.dram_tensor("cop_out", [P, ncols], F32, kind="ExternalOutput")
    mask_out = nc.dram_tensor("mask_out", [P, G * reps], F32, kind="ExternalOutput")

    with tile.TileContext(nc) as tc, ExitStack() as ctx:
        xpool = ctx.enter_context(tc.tile_pool(name="xpool", bufs=2))
        hpool = ctx.enter_context(tc.tile_pool(name="hpool", bufs=xh_bufs))
        bpool = ctx.enter_context(tc.tile_pool(name="bpool", bufs=bones_bufs))
        mpool = ctx.enter_context(tc.tile_pool(name="mpool", bufs=1))
        rpool = ctx.enter_context(tc.tile_pool(name="rpool", bufs=rot_bufs))
        vpool = ctx.enter_context(tc.tile_pool(name="vpool", bufs=1))
        spool = ctx.enter_context(tc.tile_pool(name="spool", bufs=1))
        psum = ctx.enter_context(tc.tile_pool(name="psum", bufs=1, space="PSUM"))

        acc = spool.tile([P, G * reps], F32)
        psum_M = psum.tile([P, 256], F32)
        psum_M2 = psum.tile([P, 128], F32, name="psum_M2") if relsq == "pe" else None
        if relsq != "act":
            nc.gpsimd.memset(acc[:], 0.0)

        xt_static = None
        if mode == "nodma":
            xt_static = spool.tile([P, FK], F32)
            nc.gpsimd.memset(xt_static[:], 0.5)

        loop_cm = tc.For_i(0, hw_loop, 1) if hw_loop > 1 else None
        if loop_cm is not None:
            loop_cm.__enter__()

        for rep in range(reps):
            for g in range(G):
                first = rep == 0 and g == 0
                last = rep == reps - 1 and g == G - 1

                xt = xt_static if mode == "nodma" else xpool.tile([P, FK], F32)
                if mode != "nodma":
                    for ch in range(dma_chunks):
                        lo = FK * ch // dma_chunks
                        hi = FK * (ch + 1) // dma_chunks
                        nc.sync.dma_start(xt[:, lo:hi], x.ap()[g][:, lo:hi])
                if mode == "dma":
                    sink = vpool.tile([P, 2], F32, tag="sink")
                    nc.scalar.activation(sink[:], xt[:, 0:2], ACTF.Copy)
                    continue

                xh = hpool.tile([P, 3 * SK], F16)
                xtb = xt[:]
                for c in range(3):
                    src = _ap(xtb, c * CJ, [[K, 5], [4 * K, 5], [1, K]])
                    dst = xh[:, c * SK:(c + 1) * SK].rearrange(
                        "p (jj f k) -> p jj f k", jj=5, f=5, k=K)
                    nc.scalar.activation(dst, src, ACTF.Copy)

                nslab = 5 if cross == "fused" else 3
                bones = bpool.tile([P, nslab * CB], F16, tag="bones")
                if bones_fused:
                    sd = [[SK, 3], [1, CB]]
                    nc.vector.tensor_sub(
                        bones[:, 0:3 * CB].rearrange("p (c e) -> p c e", c=3, e=CB),
                        _ap(xh[:], S5, sd), _ap(xh[:], 0, sd))
                else:
                    for s in range(3):
                        nc.vector.tensor_sub(
                            bones[:, s * CB:(s + 1) * CB],
                            xh[:, s * SK + S5: s * SK + SK],
                            xh[:, s * SK: s * SK + CB])

                bb = bones[:]
                m1 = mpool.tile([P, MR], F16, tag="m1")
                m2 = mpool.tile([P, MR], F16, tag="m2")
                rot = rpool.tile([P, MR], F16, tag="rot")
                if cross == "fused":
                    if copy34_eng == "scalar":
                        nc.scalar.copy(bones[:, 3 * CB:5 * CB], bones[:, 0:2 * CB])
                    else:
                        nc.vector.tensor_copy(bones[:, 3 * CB:5 * CB],
                                              bones[:, 0:2 * CB])
                    cdims = [[CB, 3], [S5, 3], [1, S5]]
                    nc.vector.tensor_mul(
                        m1[:].rearrange("p (c q e) -> p c q e", c=3, q=3, e=S5),
                        _ap(bb, CB + S5, cdims), _ap(bb, 2 * CB, cdims))
                    nc.vector.tensor_mul(
                        m2[:].rearrange("p (c q e) -> p c q e", c=3, q=3, e=S5),
                        _ap(bb, 2 * CB + S5, cdims), _ap(bb, CB, cdims))
                else:
                    qd = [[S5, 3], [1, S5]]
                    for c in range(3):
                        c1, c2 = (c + 1) % 3, (c + 2) % 3
                        nc.vector.tensor_mul(
                            m1[:, c * 3 * S5:(c + 1) * 3 * S5].rearrange(
                                "p (q e) -> p q e", q=3, e=S5),
                            _ap(bb, c1 * CB + S5, qd), _ap(bb, c2 * CB, qd))
                    for c in range(3):
                        c1, c2 = (c + 1) % 3, (c + 2) % 3
                        nc.vector.tensor_mul(
                            m2[:, c * 3 * S5:(c + 1) * 3 * S5].rearrange(
                                "p (q e) -> p q e", q=3, e=S5),
                            _ap(bb, c2 * CB + S5, qd), _ap(bb, c1 * CB, qd))
                nc.vector.tensor_sub(rot[:], m1[:], m2[:])

                rb = rot[:]
                for c in range(3):
                    for jb in range(NB):
                        lhsT = bones[:, c * CB + 3 * S5 + jb * 128:
                                     c * CB + 3 * S5 + (jb + 1) * 128]
                        rhs = _ap(rb, c * 3 * S5 + jb * 128, [[S5, 2], [1, 128]])
                        nc.tensor.matmul(
                            psum_M[:], lhsT, rhs,
                            start=(first and c == 0 and jb == 0),
                            stop=(last and c == 2 and jb == NB - 1))

                pp = vpool.tile([P, PPN], F16, tag="pp", bufs=1)
                nc.vector.tensor_mul(
                    pp[:].rearrange("p (h c e) -> p h c e", h=2, c=3, e=S5),
                    _ap(rb, 2 * S5, [[-2 * S5, 2], [3 * S5, 3], [1, S5]]),
                    _ap(rb, S5, [[0, 2], [3 * S5, 3], [1, S5]]))

                pb = pp[:]
                vt = vpool.tile([P, NV], F16, tag="vt")
                v = vpool.tile([P, NV], F16, tag="v", bufs=1)
                hdims = [[3 * S5, 2], [1, S5]]
                v2d = lambda t: t[:].rearrange("p (h e) -> p h e", h=2, e=S5)
                nc.vector.tensor_add(v2d(vt), _ap(pb, 0, hdims), _ap(pb, S5, hdims))
                nc.vector.tensor_add(v2d(v), v2d(vt), _ap(pb, 2 * S5, hdims))

                col = rep * G + g
                if relsq == "act":
                    mrelu = vpool.tile([P, NV], F16, tag="mrelu")
                    sqj = vpool.tile([P, NV], F16, tag="sqj")
                    nc.scalar.activation(mrelu[:], v[:], ACTF.Relu, scale=-1.0)
                    nc.scalar.activation(sqj[:], mrelu[:], ACTF.Square,
                                         accum_out=acc[:, col:col + 1])
                elif relsq == "dve":
                    mrelu = vpool.tile([P, NV], F16, tag="mrelu")
                    sqj = vpool.tile([P, NV], F16, tag="sqj")
                    nc.vector.tensor_scalar(mrelu[:], v[:], -1.0, 0.0,
                                            AluOpType.mult, AluOpType.max)
                    nc.vector.tensor_tensor_reduce(
                        sqj[:], mrelu[:], mrelu[:], 1.0, 0.0,
                        AluOpType.mult, AluOpType.add,
                        accum_out=acc[:, col:col + 1])
                else:  # pe
                    mrelu = vpool.tile([P, NV], F16, tag="mrelu", bufs=2)
                    nc.vector.tensor_scalar(mrelu[:], v[:], -1.0, 0.0,
                                            AluOpType.mult, AluOpType.max)
                    for vb in range(NVB):
                        blk = mrelu[:, vb * 128:(vb + 1) * 128]
                        nc.tensor.matmul(
                            psum_M2[:], blk, blk,
                            start=(first and vb == 0),
                            stop=(last and vb == NVB - 1))

        if loop_cm is not None:
            loop_cm.__exit__(None, None, None)

        cop_sb = spool.tile([P, ncols], F32)
        if mode != "dma":
            nc.scalar.copy(cop_sb[:, 0:256], psum_M[:])
            if psum_M2 is not None:
                nc.scalar.copy(cop_sb[:, 256:384], psum_M2[:])
        else:
            nc.gpsimd.memset(cop_sb[:], 0.0)
        nc.sync.dma_start(cop_out.ap(), cop_sb[:])
        nc.sync.dma_start(mask_out.ap(), acc[:])

    nc.compile()
    return nc, G


def host_planarize63(x: np.ndarray, n_cores: int, K: int) -> np.ndarray:
    B = x.shape[0]
    R = B // n_cores
    G = R // (P * K)
    xr = x.reshape(n_cores, G, P, K, 21, 3)
    xp = xr.transpose(0, 1, 2, 5, 4, 3)
    return np.ascontiguousarray(xp).reshape(n_cores, G, P, 63 * K)


_CACHE = {}


def _get_nc(rows_per_core: int, K: int):
    key = (rows_per_core, K)
    if key not in _CACHE:
        _CACHE[key] = build_v7(rows_per_core, K)
    return _CACHE[key]


def kernel(pose23d_pred: np.ndarray) -> np.ndarray:
    x = np.asarray(pose23d_pred, dtype=np.float32)
    assert x.shape == (B_FULL, 21, 3), x.shape
    K = 128
    R = B_FULL // N_CORES
    nc, G = _get_nc(R, K)
    xp = host_planarize63(x, N_CORES, K)
    in_maps = [{"x": xp[i]} for i in range(N_CORES)]
    res = run_bass_kernel_spmd(nc, in_maps, list(range(N_CORES)))
    total = 0.0
    for r in res.results:
        M = r["cop_out"].astype(np.float64)
        total += np.trace(M[:, 0:128]) + np.trace(M[:, 128:256])
        if M.shape[1] >= 384:
            total += np.trace(M[:, 256:384])
        total += r["mask_out"].astype(np.float64).sum()
    return np.float32(total)
